# revision 16
# baseline (speedup 1.0000x reference)
"""DiceLoss (softmax + one-hot gather + per-sample dice) on 8 trn2 cores.

Sharding: pure data-parallel over the batch dim (N=32 -> 4 samples/core).
Each core streams its 4 samples, computing per-pixel
    p = exp(x_t) / sum_c exp(x_c)
and accumulating per-partition sums of p. The host finishes with the
(tiny) dice formula. The softmax prob sum over classes is identically 1
per pixel, so cardinality = 2*H*W analytically (matches the reference's
jnp.sum(probs) + H*W to ~1e-7 relative).

Per-core layout: partitions = (4 samples x 32 pixel-blocks) = 128; free
dim = 8192 pixels per block, processed in 4 chunks of 2048.

Engines:
  - DMA: x via HWDGE fp32 (4x 1MiB/chunk), t via SWDGE with int32->bf16 cast
  - ACT: exp per class (bf16 out), then 1/denom as exp(-ln(denom))
  - DVE: fused (t==c)*e_c via scalar_tensor_tensor, add trees,
         final mult+reduce via tensor_tensor_reduce (accum_out)
  - PE/GPSIMD: idle (memory-bound problem)
"""

import os
import sys

import numpy as np


def _ensure_concourse():
    try:
        import concourse.bass  # noqa: F401
    except ImportError:
        for p in (
            "/opt/trn_rl_repo",
            os.path.expanduser("~/.axon_site/_ro/trn_rl_repo"),
        ):
            if os.path.isdir(p) and p not in sys.path:
                sys.path.insert(0, p)


_ensure_concourse()

import concourse.bacc as bacc  # noqa: E402
import concourse.bass as bass  # noqa: E402
import concourse.mybir as mybir  # noqa: E402
from concourse.bass_utils import run_bass_kernel_spmd  # noqa: E402
from concourse.tile import TileContext  # noqa: E402
from concourse.tile_rust import add_dep_helper  # noqa: E402

N, C, H, W = 32, 4, 512, 512
NCORES = 8
SPC = N // NCORES  # samples per core = 4
PB = 32  # pixel blocks per sample (partition sub-dim)
P = SPC * PB  # 128 partitions
FTOT = H * W // PB  # 8192 free-dim pixels per block
FC = 2048  # chunk size along free dim
NCHUNK = FTOT // FC  # 4
EPS = 1e-6

_cache = {}
LAST_EXEC_NS = None


def _build():
    nc = bacc.Bacc(None)
    x = nc.dram_tensor("x", [SPC, C, H, W], mybir.dt.float32, kind="ExternalInput")
    t = nc.dram_tensor("t", [SPC, 1, H, W], mybir.dt.int32, kind="ExternalInput")
    out = nc.dram_tensor("out", [P, NCHUNK], mybir.dt.float32, kind="ExternalOutput")

    # pixel index = (pb*16 + fh)*W + w ; partition = (s, pb); free = (fh, w)
    xv = x[:].rearrange("s c (pb fh) w -> c s pb (fh w)", pb=PB)  # [4, 4, 32, 8192]
    tv = t[:].rearrange("s o (pb fh) w -> (s o) pb (fh w)", pb=PB)  # [4, 32, 8192]

    AF = mybir.ActivationFunctionType
    OP = mybir.AluOpType
    f32 = mybir.dt.float32
    bf16 = mybir.dt.bfloat16

    with TileContext(nc) as tc:
        with (
            tc.tile_pool(name="accp", bufs=1) as accp,
            tc.tile_pool(name="xp", bufs=2) as xp,
            tc.tile_pool(name="ep", bufs=2) as ep,
            tc.tile_pool(name="tp", bufs=2) as tp,
            tc.tile_pool(name="up", bufs=2) as up,
        ):
            accs = [
                accp.tile([P, 1], f32, tag=f"acc{k}", name=f"acc{k}")
                for k in range(NCHUNK)
            ]
            for k in range(NCHUNK):
                sl = slice(k * FC, (k + 1) * FC)
                X = [
                    xp.tile([P, FC], f32, tag=f"x{c}", name=f"X{c}_{k}")
                    for c in range(C)
                ]
                E = [
                    ep.tile([P, FC], bf16, tag=f"e{c}", name=f"E{c}_{k}")
                    for c in range(C)
                ]
                U = [
                    up.tile([P, FC], bf16, tag=f"u{c}", name=f"U{c}_{k}")
                    for c in range(C)
                ]
                D1 = up.tile([P, FC], bf16, tag="d1", name=f"D1_{k}")
                D2 = up.tile([P, FC], bf16, tag="d2", name=f"D2_{k}")
                T = tp.tile([P, FC], mybir.dt.int32, tag="t", name=f"T_{k}")
                T2 = up.tile([P, FC], bf16, tag="t2", name=f"T2_{k}")

                for c in range(C):
                    # issue from the ACT sequencer (also HWDGE): the X-slot
                    # WAR release is vs ACT's exp read, so it becomes a
                    # same-engine ordering instead of a second sync wait
                    nc.scalar.dma_start(X[c][:], xv[c, :, :, sl])
                nc.sync.dma_start(T[:], tv[:, :, sl])

                for c in range(C):
                    nc.scalar.activation(E[c][:], X[c][:], AF.Exp)

                # absorb the T-DMA wait into a TT op (which also converts
                # int32 -> bf16); the denom tree absorbs the ACT waits; the
                # STT gathers scheduled after them then need no sync waits
                # (TensorScalarPtr has 1 wait slot).
                nc.vector.tensor_tensor(T2[:], T[:], T[:], OP.max)
                nc.vector.tensor_tensor(D1[:], E[0][:], E[1][:], OP.add)
                nc.vector.tensor_tensor(D2[:], E[2][:], E[3][:], OP.add)
                i_d3 = nc.vector.tensor_tensor(D1[:], D1[:], D2[:], OP.add)
                # u_c = (t == c) * e_c, fused gather
                for c in range(C):
                    i_stt = nc.vector.scalar_tensor_tensor(
                        U[c][:], T2[:], float(c), E[c][:], OP.is_equal, OP.mult
                    )
                    add_dep_helper(
                        i_stt.ins, i_d3.ins, False, "order STT after denom tree"
                    )
                # numer = e_t
                nc.vector.tensor_tensor(U[0][:], U[0][:], U[1][:], OP.add)
                nc.vector.tensor_tensor(U[2][:], U[2][:], U[3][:], OP.add)
                nc.vector.tensor_tensor(U[0][:], U[0][:], U[2][:], OP.add)
                # p = numer/denom in log domain: exp(ln(numer) - ln(denom));
                # the final exp carries accum_out = per-partition sum of p.
                # Distinct output tiles keep every ACT instr at <=1 sync wait.
                nc.scalar.activation(D1[:], D1[:], AF.Ln)
                nc.scalar.activation(U[1][:], U[0][:], AF.Ln)
                nc.vector.tensor_tensor(U[1][:], U[1][:], D1[:], OP.subtract)
                nc.scalar.activation(
                    U[2][:], U[1][:], AF.Exp, accum_out=accs[k][:]
                )
            for k in range(NCHUNK):
                nc.scalar.dma_start(out[:, k : k + 1], accs[k][:])
    nc.compile()  # bacc passes: split sync waits, fill ISA bytes, ...
    return nc


def kernel(input, target):
    global LAST_EXEC_NS
    nc = _cache.get("nc")
    if nc is None:
        nc = _cache.setdefault("nc", _build())

    input = np.asarray(input)
    target = np.asarray(target)
    in_maps = []
    for i in range(NCORES):
        in_maps.append(
            {
                "x": np.ascontiguousarray(
                    input[i * SPC : (i + 1) * SPC], dtype=np.float32
                ),
                "t": np.ascontiguousarray(
                    target[i * SPC : (i + 1) * SPC], dtype=np.int32
                ),
            }
        )
    res = run_bass_kernel_spmd(nc, in_maps, list(range(NCORES)))
    LAST_EXEC_NS = res.exec_time_ns

    Is = []
    for i in range(NCORES):
        o = np.asarray(res.results[i]["out"], dtype=np.float64)  # [128, NCHUNK]
        Is.append(o.sum(axis=1).reshape(SPC, PB).sum(axis=1))
    intersection = np.concatenate(Is)  # [32]
    hw = float(H * W)
    dice = 2.0 * intersection / (hw + hw + EPS)
    return np.float32(np.mean(1.0 - dice))
